# revision 1
# baseline (speedup 1.0000x reference)
"""Trainium2 Bass kernel for ComputeNodeAreaFromRouteMap (DREAMPlace-style
weighted-overlap map sampling).

area_i = sum_{a,b} ovx[i,a] * ovy[i,b] * U[bx0_i+a, by0_i+b]

Strategy: host expands the 512x512 map into a window table WT[qx*256+hy] =
U[4qx:4qx+6, 2hy:2hy+4] (6x4 f32 slab, 96B payload in a 256B-stride row, 15-bit
index fits the dma_gather int16 index format). Device: per node compute the
record index + 6/4-tap overlap weights (clamp differences of the fractional
coordinates), gather one record per node via the GPSIMD dma_gather ucode
(1024-idx sub-calls, 4 SWDGE queues), and reduce W . (ovx x ovy) on DVE.
Data-parallel over nodes across the 8 NeuronCores; the table is replicated.
"""
import numpy as np

import concourse.bacc as bacc
import concourse.bass as bass
import concourse.tile as tile
import concourse.mybir as mybir
from concourse import bass_utils
from concourse import ap_utils
from concourse._compat import exact_div

# ---- problem constants (hardcoded per the task contract) ----
XL, YL, XH, YH = 0.0, 0.0, 1000.0, 1000.0
NUM_MOVABLE = 1_000_000
NBX, NBY = 512, 512
BSX = (XH - XL) / NBX            # 1.953125
BSY = (YH - YL) / NBY
INV_BSX = 1.0 / BSX
INV_BSY = 1.0 / BSY

NCORES = 8
P = 128
NPP = 1024                        # cols per partition per core
NPC = P * NPP                     # 131072 padded nodes per core
CHUNK = 128                       # cols per chunk
NCHUNK = NPP // CHUNK             # 8
SUBC = 8                          # cols per gather sub-call (1024 indices)
NSUB = CHUNK // SUBC              # 16 sub-calls per chunk
NW = 32768                        # window-table rows (128 qx * 256 hy)
ESIZE = 24                        # 6 rows x 4 cols window payload (f32)
ESTEP = 64                        # table row stride in elements (256B)

f32 = mybir.dt.float32
i16 = mybir.dt.int16
i32 = mybir.dt.int32

AL = mybir.AluOpType
AX = mybir.AxisListType

# Pin each dma_gather's DMA-completion sem lane to its SWDGE queue so lanes
# never mix queues regardless of the scheduler's instruction interleaving
# (the sim's per-lane queue lock models real FIFO-order hazards).
import concourse.tile_sem_assignment as _tsa

if not getattr(_tsa, "_ant_gather_lane_patch", False):
    _orig_assign_tick = _tsa.TileClockTick._assign_tick

    def _patched_assign_tick(self, inst):
        if isinstance(inst, mybir.InstDMAGatherAnt):
            self.next_sw_dma_idx = inst.queue_num
        return _orig_assign_tick(self, inst)

    _tsa.TileClockTick._assign_tick = _patched_assign_tick
    _tsa._ant_gather_lane_patch = True


def _emit_dma_gather(nc, out_ap, in_ap, idxs_ap, num_idxs, elem_size, elem_step,
                     queue_num):
    """bass.dma_gather without the elem_size%256 restriction (256B granularity
    constrains the table row *stride*, not the payload length)."""
    gp = nc.gpsimd
    stride_bytes = elem_step * mybir.dt.size(in_ap.dtype)
    stride_bytes_256 = exact_div(stride_bytes, 256)
    assert idxs_ap.dtype == i16
    assert in_ap.ap[0][0] == elem_step and in_ap.ap[-1][1] == elem_size
    assert out_ap.ap[-1][1] == elem_size
    assert out_ap.ap[0][1] * out_ap.ap[1][1] == num_idxs
    assert num_idxs % 128 == 0
    assert ap_utils.ap_is_contiguous(out_ap.ap[2:])
    assert ap_utils.ap_is_contiguous(idxs_ap.ap[1:])

    _in_ap = gp.lower_ap_dma(in_ap, for_custom_bir_dma=True)
    _idxs_ap = gp.lower_ap(idxs_ap)
    _out_ap = gp.lower_ap(out_ap)
    return gp.add_instruction(
        mybir.InstDMAGatherAnt(
            name=nc.get_next_instruction_name(),
            ins=[*_in_ap, _idxs_ap, gp.lower_val_access(gp.to_reg(num_idxs))],
            outs=[_out_ap],
            transpose=False,
            num_idxs=num_idxs,
            elem_size=elem_size,
            stride_bytes_256=stride_bytes_256,
            gen_mode=0,
            single_packet=True,
            queue_num=queue_num,
            sbuf_tokens_per_rank=0,
            sbuf_free_dim_per_rank=0,
            sbuf_free_dim_pad_per_rank=0,
            sbuf_byte_offset=0,
        )
    )


def _axis_prep(nc, pool, pos, size, inv_bs, shift, tag):
    """Per-axis: exact floor bin, aligned base, fractional coords.

    Returns (q_i32 tile [P,CHUNK] of bin>>shift, flo tile, fhi tile).
    """
    v = nc.vector
    z = pool.tile([P, CHUNK], f32, tag=f"{tag}z")
    zh = pool.tile([P, CHUNK], f32, tag=f"{tag}zh")
    t0 = pool.tile([P, CHUNK], f32, tag=f"{tag}t0")
    v.tensor_scalar(z[:], pos[:], inv_bs, None, AL.mult)        # z = pos/bs
    v.tensor_scalar(t0[:], size[:], inv_bs, None, AL.mult)      # t0 = size/bs
    v.tensor_add(zh[:], z[:], t0[:])                            # zh = z + t0

    bi = pool.tile([P, CHUNK], i32, tag=f"{tag}bi")
    bf = pool.tile([P, CHUNK], f32, tag=f"{tag}bf")
    gt = pool.tile([P, CHUNK], f32, tag=f"{tag}gt")
    v.tensor_copy(bi[:], z[:])                                  # round-nearest
    v.tensor_copy(bf[:], bi[:])
    v.tensor_tensor(gt[:], bf[:], z[:], AL.is_gt)               # 1.0 if bf > z
    v.tensor_sub(bf[:], bf[:], gt[:])                           # exact floor
    v.tensor_scalar(bf[:], bf[:], 0.0, 509.0, AL.max, AL.min)   # clip bin

    q = pool.tile([P, CHUNK], i32, tag=f"{tag}q")
    base_i = pool.tile([P, CHUNK], i32, tag=f"{tag}basei")
    base_f = pool.tile([P, CHUNK], f32, tag=f"{tag}basef")
    v.tensor_copy(q[:], bf[:])                                  # exact int
    v.tensor_scalar(q[:], q[:], shift, None, AL.arith_shift_right)
    v.tensor_scalar(base_i[:], q[:], shift, None, AL.logical_shift_left)
    v.tensor_copy(base_f[:], base_i[:])

    flo = pool.tile([P, CHUNK], f32, tag=f"{tag}flo")
    fhi = pool.tile([P, CHUNK], f32, tag=f"{tag}fhi")
    v.tensor_sub(flo[:], z[:], base_f[:])
    v.tensor_sub(fhi[:], zh[:], base_f[:])
    return q, flo, fhi


def _weights(nc, pool, iota, flo, fhi, ntap, tag):
    """ov[a] = clamp(fhi - a, 0, 1) - clamp(flo - a, 0, 1), a = 0..ntap-1.

    Returns tile [P, CHUNK*ntap] (node-major, tap-minor)."""
    v = nc.vector
    d1 = pool.tile([P, CHUNK, ntap], f32, tag=f"{tag}d1")
    d2 = pool.tile([P, CHUNK, ntap], f32, tag=f"{tag}d2")
    ov = pool.tile([P, CHUNK, ntap], f32, tag=f"{tag}ov")
    iota_b = iota[:, 0:ntap].unsqueeze(1).to_broadcast([P, CHUNK, ntap])
    v.tensor_tensor(d1[:], fhi[:].unsqueeze(2).to_broadcast([P, CHUNK, ntap]),
                    iota_b, AL.subtract)
    v.tensor_scalar(d1[:], d1[:], 0.0, 1.0, AL.max, AL.min)
    v.tensor_tensor(d2[:], flo[:].unsqueeze(2).to_broadcast([P, CHUNK, ntap]),
                    iota_b, AL.subtract)
    v.tensor_scalar(d2[:], d2[:], 0.0, 1.0, AL.max, AL.min)
    v.tensor_sub(ov[:], d1[:], d2[:])
    return ov


def build(repeat=1, num_cores=NCORES):
    nc = bacc.Bacc(None, target_bir_lowering=False, debug=False,
                   num_swdge_queues=4)

    x_in = nc.dram_tensor("x_in", [NPC], f32, kind="ExternalInput")
    y_in = nc.dram_tensor("y_in", [NPC], f32, kind="ExternalInput")
    sx_in = nc.dram_tensor("sx_in", [NPC], f32, kind="ExternalInput")
    sy_in = nc.dram_tensor("sy_in", [NPC], f32, kind="ExternalInput")
    wt_in = nc.dram_tensor("wt_in", [NW, ESTEP], f32, kind="ExternalInput")
    xw_in = nc.dram_tensor("xw_in", [P * NPP * 8], f32, kind="ExternalInput")
    yw_in = nc.dram_tensor("yw_in", [P * NPP * 8], f32, kind="ExternalInput")
    area_out = nc.dram_tensor("area_out", [NPC], f32, kind="ExternalOutput")

    wt_gather_ap = bass.AP(wt_in[:].tensor, 0, [[ESTEP, NW], [1, ESIZE]])
    # node id i = c*128 + p  ->  tile position (p, c)
    x_t = x_in[:].rearrange("(c p) -> p c", p=P)
    y_t = y_in[:].rearrange("(c p) -> p c", p=P)
    sx_t = sx_in[:].rearrange("(c p) -> p c", p=P)
    sy_t = sy_in[:].rearrange("(c p) -> p c", p=P)
    out_t = area_out[:].rearrange("(c p) -> p c", p=P)
    xw_t = xw_in[:].rearrange("(p s) -> p s", p=P)
    yw_t = yw_in[:].rearrange("(p s) -> p s", p=P)

    with tile.TileContext(nc) as tc:
        with (
            tc.tile_pool(name="const", bufs=1) as cpool,
            tc.tile_pool(name="work", bufs=2) as pool,
            tc.tile_pool(name="wwin", bufs=3) as wpool,
            tc.tile_pool(name="idxp", bufs=2) as ipool,
        ):
            iota = cpool.tile([P, 6], f32)
            for k in range(6):
                nc.vector.memset(iota[:, k:k + 1], float(k))

            def body():
                for ch in range(NCHUNK):
                    cs = slice(ch * CHUNK, (ch + 1) * CHUNK)
                    x = pool.tile([P, CHUNK], f32, tag="x")
                    y = pool.tile([P, CHUNK], f32, tag="y")
                    sx = pool.tile([P, CHUNK], f32, tag="sx")
                    sy = pool.tile([P, CHUNK], f32, tag="sy")
                    nc.sync.dma_start(x[:], x_t[:, cs])
                    nc.sync.dma_start(y[:], y_t[:, cs])
                    nc.sync.dma_start(sx[:], sx_t[:, cs])
                    nc.sync.dma_start(sy[:], sy_t[:, cs])

                    _, fxl, fxh = _axis_prep(nc, pool, x, sx, INV_BSX, 2, "x")
                    _, fyl, fyh = _axis_prep(nc, pool, y, sy, INV_BSY, 1, "y")

                    # index chain, computed directly in the dma_gather wrapped
                    # layout from host-prearranged (replicated) x/y copies
                    WC = CHUNK * 8
                    ws = slice(ch * WC, (ch + 1) * WC)
                    xw = ipool.tile([P, WC], f32, tag="xw")
                    yw = ipool.tile([P, WC], f32, tag="yw")
                    nc.sync.dma_start(xw[:], xw_t[:, ws])
                    nc.sync.dma_start(yw[:], yw_t[:, ws])

                    def wrapped_bin(pos_t, inv_bs, shift, tg):
                        v = nc.vector
                        ia = ipool.tile([P, WC], i32, tag=f"{tg}ia")
                        fb = ipool.tile([P, WC], f32, tag=f"{tg}fb")
                        gtw = ipool.tile([P, WC], f32, tag=f"{tg}gt")
                        v.tensor_scalar(pos_t[:], pos_t[:], inv_bs, None,
                                        AL.mult)
                        v.tensor_copy(ia[:], pos_t[:])       # round-nearest
                        nc.scalar.copy(fb[:], ia[:])
                        v.tensor_tensor(gtw[:], fb[:], pos_t[:], AL.is_gt)
                        v.tensor_sub(fb[:], fb[:], gtw[:])   # exact floor
                        v.tensor_copy(ia[:], fb[:])
                        v.tensor_scalar(ia[:], ia[:], shift, None,
                                        AL.arith_shift_right)
                        return ia

                    qxw = wrapped_bin(xw, INV_BSX, 2, "qx")
                    hyw = wrapped_bin(yw, INV_BSY, 1, "hy")
                    flat = ipool.tile([P, WC], i32, tag="flat")
                    nc.vector.scalar_tensor_tensor(
                        out=flat[:], in0=qxw[:], scalar=256, in1=hyw[:],
                        op0=AL.mult, op1=AL.add)
                    idxt = ipool.tile([P, WC], i16, tag="idxt")
                    nc.vector.tensor_copy(idxt[:], flat[:])

                    # gather: NSUB sub-calls of SUBC*128 indices each,
                    # rotated across the 4 SWDGE queues
                    w = wpool.tile([P, CHUNK * ESIZE], f32, tag="w")
                    for j in range(NSUB):
                        _emit_dma_gather(
                            nc,
                            w[:, j * SUBC * ESIZE:(j + 1) * SUBC * ESIZE]
                            .rearrange("p (c e) -> p c e", e=ESIZE),
                            wt_gather_ap,
                            idxt[:, j * SUBC * 8:(j + 1) * SUBC * 8],
                            SUBC * P, ESIZE, ESTEP, queue_num=j % 4,
                        )

                    ovx = _weights(nc, pool, iota, fxl, fxh, 6, "wx")
                    ovy = _weights(nc, pool, iota, fyl, fyh, 4, "wy")

                    # m[p,c,a,b] = W * ovy[b];  t = sum_b;  s = t * ovx;
                    # area = sum_a * (BSX*BSY)
                    m = w[:].rearrange("p (c a b) -> p c a b", a=6, b=4)
                    ovy_b = ovy[:].unsqueeze(2).to_broadcast([P, CHUNK, 6, 4])
                    nc.vector.tensor_tensor(m, m, ovy_b, AL.mult)
                    t = pool.tile([P, CHUNK, 6], f32, tag="t")
                    nc.vector.tensor_reduce(t[:], m, AX.X, AL.add)
                    nc.vector.tensor_tensor(t[:], t[:], ovx[:], AL.mult)
                    area = pool.tile([P, CHUNK], f32, tag="area")
                    nc.vector.tensor_reduce(
                        area[:], t[:].rearrange("p c a -> p c a"), AX.X, AL.add)
                    nc.vector.tensor_scalar(area[:], area[:], BSX * BSY, None,
                                            AL.mult)
                    nc.sync.dma_start(out_t[:, cs], area[:])

            if repeat == 1:
                body()
            else:
                with tc.For_i(0, repeat, 1):
                    body()

    nc.compile()
    return nc


def make_window_table(utilization_map):
    U = np.asarray(utilization_map, np.float32)
    Upad = np.zeros((520, 520), np.float32)
    Upad[:512, :512] = U
    # WT[qx*256+hy, a*4+b] = Upad[4qx+a, 2hy+b]
    a = np.arange(6)
    b = np.arange(4)
    qx = np.arange(128)
    hy = np.arange(256)
    rows = (4 * qx[:, None, None, None] + a[None, None, :, None])     # [128,1,6,1]
    cols = (2 * hy[None, :, None, None] + b[None, None, None, :])     # [1,256,1,4]
    win = Upad[rows, cols]                                            # [128,256,6,4]
    wt = np.zeros((NW, ESTEP), np.float32)
    wt[:, :ESIZE] = win.reshape(NW, ESIZE)
    return wt


def make_in_maps(pos, node_size_x, node_size_y, utilization_map):
    n = NUM_MOVABLE
    half = pos.shape[0] // 2
    x = np.asarray(pos[:n], np.float32)
    y = np.asarray(pos[half:half + n], np.float32)
    sx = np.asarray(node_size_x, np.float32)
    sy = np.asarray(node_size_y, np.float32)

    tot = NCORES * NPC
    xp = np.full(tot, 500.0, np.float32)
    yp = np.full(tot, 500.0, np.float32)
    sxp = np.full(tot, 0.5, np.float32)
    syp = np.full(tot, 0.5, np.float32)
    xp[:n] = x
    yp[:n] = y
    sxp[:n] = sx
    syp[:n] = sy

    wt = make_window_table(utilization_map)

    def wrapped(arr_core):
        # value for tile (p = 16g + r, s = (ch*128 + m)*8 + a)
        #   = arr[ch*16384 + m*128 + 16a + r]   (replicated over g)
        v = arr_core.reshape(NCHUNK, CHUNK, 8, 16)       # [ch, m, a, r]
        v = v.transpose(3, 0, 1, 2).reshape(16, NPP * 8)  # [r, ch*m*a]
        return np.tile(v, (8, 1)).reshape(-1).copy()

    in_maps = []
    for k in range(NCORES):
        s = slice(k * NPC, (k + 1) * NPC)
        in_maps.append(dict(x_in=xp[s], y_in=yp[s], sx_in=sxp[s], sy_in=syp[s],
                            xw_in=wrapped(xp[s]), yw_in=wrapped(yp[s]),
                            wt_in=wt))
    return in_maps


_NC_CACHE = {}


def _get_nc(repeat=1):
    if repeat not in _NC_CACHE:
        _NC_CACHE[repeat] = build(repeat)
    return _NC_CACHE[repeat]


def kernel(pos, node_size_x, node_size_y, utilization_map):
    in_maps = make_in_maps(pos, node_size_x, node_size_y, utilization_map)
    nc = _get_nc(1)
    res = bass_utils.run_bass_kernel_spmd(nc, in_maps,
                                          core_ids=list(range(NCORES)))
    outs = [np.asarray(r["area_out"]) for r in res.results]
    area = np.concatenate(outs)[:NUM_MOVABLE]
    return area.astype(np.float32)



# revision 6
# speedup vs baseline: 1.5828x; 1.5828x over previous
"""Trainium2 Bass kernel for ComputeNodeAreaFromRouteMap (DREAMPlace-style
weighted-overlap map sampling).

area_i = sum_{a,b} ovx[i,a] * ovy[i,b] * U[bx0_i+a, by0_i+b]

Strategy: host expands the 512x512 map into an fp16 window table
WT[qx*256+hy] = U[4qx-1:4qx+6, 2hy-1:2hy+4] (7x5 window, 70B payload in
a 256B-stride row; 15-bit index fits the dma_gather int16 index format).
The window starts one bin early because the f32->i32 bin conversion
truncates in CoreSim but rounds-to-nearest on HW; taps -1..5 cover both
conventions and the clamp weights zero out the unused taps.
Device, per 131072-node pass:
  - weight path: x/y/sx/sy in natural [128, 1024] layout (host
    pre-transposed so every DMA is contiguous), trunc-based floor
    (f32->i32 copy truncates; z >= 0), fp16 clamp-difference weights
    computed once per pass.
  - index path (per 16384-node chunk): x,y loaded in compact wrapped
    [16, 1024] layout (host permuted), bit-identical floor chain ->
    int16 record indices, replicated to all 8 GPSIMD core groups.
  - gather: 16 x 1024-index dma_gather sub-calls (4 SWDGE queues) fetch
    one 24-fp16 record per node; this ucode is the serial backbone
    (~2.5 ns/idx).
  - reduce: fp16 multiply-reduce against the weights, f32 output.
Data-parallel over nodes across the 8 NeuronCores; the table is
replicated. All host-side work is pure data reordering.
"""
import numpy as np

import concourse.bacc as bacc
import concourse.bass as bass
import concourse.tile as tile
import concourse.mybir as mybir
from concourse import bass_utils
from concourse import ap_utils
from concourse._compat import exact_div

# ---- problem constants (hardcoded per the task contract) ----
XL, YL, XH, YH = 0.0, 0.0, 1000.0, 1000.0
NUM_MOVABLE = 1_000_000
NBX, NBY = 512, 512
BSX = (XH - XL) / NBX            # 1.953125
BSY = (YH - YL) / NBY
INV_BSX = 1.0 / BSX
INV_BSY = 1.0 / BSY

NCORES = 8
P = 128
NPP = 1024                        # cols per partition per core
NPC = P * NPP                     # 131072 padded nodes per core
CHUNK = 128                       # cols per chunk
NCHUNK = NPP // CHUNK             # 8
SUBC = 8                          # cols per gather sub-call (1024 indices)
NSUB = CHUNK // SUBC              # 16 sub-calls per chunk
NW = 32768                        # window-table rows (128 qx * 256 hy)
ESIZE = 35                        # 7 rows x 5 cols window payload (fp16)
ESTEP = 128                       # table row stride in fp16 elements (256B)
NTAPX = 7                         # tap offsets -1..5 (base may be floor+1:
NTAPY = 5                         # f32->i32 copy truncates in CoreSim but
                                  # rounds on HW; the wide window covers both)

f32 = mybir.dt.float32
f16 = mybir.dt.float16
i16 = mybir.dt.int16
i32 = mybir.dt.int32

AL = mybir.AluOpType
AX = mybir.AxisListType

# Pin each dma_gather's DMA-completion sem lane to its SWDGE queue so lanes
# never mix queues regardless of the scheduler's instruction interleaving
# (the sim's per-lane queue lock models real FIFO-order hazards).
import concourse.tile_sem_assignment as _tsa

if not getattr(_tsa, "_ant_gather_lane_patch", False):
    _orig_assign_tick = _tsa.TileClockTick._assign_tick

    def _patched_assign_tick(self, inst):
        if isinstance(inst, mybir.InstDMAGatherAnt):
            self.next_sw_dma_idx = inst.queue_num
        return _orig_assign_tick(self, inst)

    _tsa.TileClockTick._assign_tick = _patched_assign_tick
    _tsa._ant_gather_lane_patch = True


def _emit_dma_gather(nc, out_ap, in_ap, idxs_ap, num_idxs, elem_size, elem_step,
                     queue_num):
    """bass.dma_gather without the elem_size%256 restriction (256B granularity
    constrains the table row *stride*, not the payload length)."""
    gp = nc.gpsimd
    stride_bytes = elem_step * mybir.dt.size(in_ap.dtype)
    stride_bytes_256 = exact_div(stride_bytes, 256)
    assert idxs_ap.dtype == i16
    assert in_ap.ap[0][0] == elem_step and in_ap.ap[-1][1] == elem_size
    assert out_ap.ap[-1][1] == elem_size
    assert out_ap.ap[0][1] * out_ap.ap[1][1] == num_idxs
    assert num_idxs % 128 == 0
    assert ap_utils.ap_is_contiguous(out_ap.ap[2:])
    assert ap_utils.ap_is_contiguous(idxs_ap.ap[1:])

    _in_ap = gp.lower_ap_dma(in_ap, for_custom_bir_dma=True)
    _idxs_ap = gp.lower_ap(idxs_ap)
    _out_ap = gp.lower_ap(out_ap)
    return gp.add_instruction(
        mybir.InstDMAGatherAnt(
            name=nc.get_next_instruction_name(),
            ins=[*_in_ap, _idxs_ap, gp.lower_val_access(gp.to_reg(num_idxs))],
            outs=[_out_ap],
            transpose=False,
            num_idxs=num_idxs,
            elem_size=elem_size,
            stride_bytes_256=stride_bytes_256,
            gen_mode=0,
            single_packet=True,
            queue_num=queue_num,
            sbuf_tokens_per_rank=0,
            sbuf_free_dim_per_rank=0,
            sbuf_free_dim_pad_per_rank=0,
            sbuf_byte_offset=0,
        )
    )


def build(repeat=1, num_cores=NCORES):
    nc = bacc.Bacc(None, target_bir_lowering=False, debug=False,
                   num_swdge_queues=4)

    x_in = nc.dram_tensor("x_in", [NPC], f32, kind="ExternalInput")
    y_in = nc.dram_tensor("y_in", [NPC], f32, kind="ExternalInput")
    sx_in = nc.dram_tensor("sx_in", [NPC], f32, kind="ExternalInput")
    sy_in = nc.dram_tensor("sy_in", [NPC], f32, kind="ExternalInput")
    wt_in = nc.dram_tensor("wt_in", [NW, ESTEP], f16, kind="ExternalInput")
    xi_in = nc.dram_tensor("xi_in", [16 * NPP * 8], f32, kind="ExternalInput")
    yi_in = nc.dram_tensor("yi_in", [16 * NPP * 8], f32, kind="ExternalInput")
    area_out = nc.dram_tensor("area_out", [NPC], f32, kind="ExternalOutput")

    wt_gather_ap = bass.AP(wt_in[:].tensor, 0, [[ESTEP, NW], [1, ESIZE]])
    # host pre-transposed: DRAM is tile-order p-major, all loads contiguous
    x_t = x_in[:].rearrange("(p c) -> p c", p=P)
    y_t = y_in[:].rearrange("(p c) -> p c", p=P)
    sx_t = sx_in[:].rearrange("(p c) -> p c", p=P)
    sy_t = sy_in[:].rearrange("(p c) -> p c", p=P)
    out_t = area_out[:].rearrange("(p c) -> p c", p=P)
    xi_t = xi_in[:].rearrange("(r s) -> r s", r=16)
    yi_t = yi_in[:].rearrange("(r s) -> r s", r=16)

    with tile.TileContext(nc) as tc:
        with (
            tc.tile_pool(name="const", bufs=1) as cpool,
            tc.tile_pool(name="inp", bufs=1) as xpool,
            tc.tile_pool(name="scr", bufs=1) as spool,
            tc.tile_pool(name="per", bufs=1) as gpool,
            tc.tile_pool(name="wwin", bufs=3) as wpool,
            tc.tile_pool(name="idxp", bufs=2) as ipool,
            tc.tile_pool(name="red", bufs=2) as rpool,
        ):
            iota = cpool.tile([P, NTAPX], f16)
            for k in range(NTAPX):
                nc.vector.memset(iota[:, k:k + 1], float(k) - 1.0)

            def axis_prep(pos, size, inv_bs, shift, tag):
                """z = pos/bs; zh = z + size/bs; b = trunc(z) == floor(z)
                (z >= 0); base = (b>>shift)<<shift; fl = z - base,
                fh = zh - base (fp16)."""
                v = nc.vector
                zm = spool.tile([P, NPP], f32, tag="zm")
                zmh = spool.tile([P, NPP], f32, tag="zmh")
                v.tensor_scalar(zm[:], pos[:], inv_bs, None, AL.mult)
                v.scalar_tensor_tensor(out=zmh[:], in0=size[:], scalar=inv_bs,
                                       in1=zm[:], op0=AL.mult, op1=AL.add)
                bi = spool.tile([P, NPP], i32, tag="bi")
                v.tensor_copy(bi[:], zm[:])     # f32->i32 truncates == floor
                base = spool.tile([P, NPP], i32, tag="base")
                v.tensor_scalar(base[:], bi[:], shift, None,
                                AL.arith_shift_right)
                v.tensor_scalar(base[:], base[:], shift, None,
                                AL.logical_shift_left)
                basef = spool.tile([P, NPP], f32, tag="basef")
                v.tensor_copy(basef[:], base[:])
                fl = gpool.tile([P, NPP], f16, tag=f"{tag}fl")
                fh = gpool.tile([P, NPP], f16, tag=f"{tag}fh")
                v.tensor_tensor(fl[:], zm[:], basef[:], AL.subtract)
                v.tensor_tensor(fh[:], zmh[:], basef[:], AL.subtract)
                return fl, fh

            def weights(fl, fh, ntap, tag):
                """ov[a] = clamp(fh - a, 0, 1) - clamp(fl - a, 0, 1)
                over the whole pass: [P, NPP, ntap] fp16."""
                v = nc.vector
                ov = gpool.tile([P, NPP, ntap], f16, tag=f"{tag}ov")
                d2 = spool.tile([P, NPP, ntap], f16, tag=f"{tag}d2")
                iota_b = iota[:, 0:ntap].unsqueeze(1).to_broadcast(
                    [P, NPP, ntap])
                v.tensor_tensor(ov[:], fh[:].unsqueeze(2).to_broadcast(
                    [P, NPP, ntap]), iota_b, AL.subtract)
                v.tensor_scalar(ov[:], ov[:], 0.0, 1.0, AL.max, AL.min)
                v.tensor_tensor(d2[:], fl[:].unsqueeze(2).to_broadcast(
                    [P, NPP, ntap]), iota_b, AL.subtract)
                v.tensor_scalar(d2[:], d2[:], 0.0, 1.0, AL.max, AL.min)
                v.tensor_sub(ov[:], ov[:], d2[:])
                return ov

            def body():
                v = nc.vector
                # ---- per-pass weight prep in natural [128, 1024] layout ----
                x = xpool.tile([P, NPP], f32, tag="x")
                y = xpool.tile([P, NPP], f32, tag="y")
                sx = xpool.tile([P, NPP], f32, tag="sx")
                sy = xpool.tile([P, NPP], f32, tag="sy")
                nc.sync.dma_start(x[:], x_t)
                nc.sync.dma_start(y[:], y_t)
                nc.sync.dma_start(sx[:], sx_t)
                nc.sync.dma_start(sy[:], sy_t)
                flx, fhx = axis_prep(x, sx, INV_BSX, 2, "x")
                fly, fhy = axis_prep(y, sy, INV_BSY, 1, "y")
                ovx = weights(flx, fhx, NTAPX, "wx")  # [P, NPP, 7]
                ovy = weights(fly, fhy, NTAPY, "wy")  # [P, NPP, 5]

                for ch in range(NCHUNK):
                    cs = slice(ch * CHUNK, (ch + 1) * CHUNK)
                    ws = slice(ch * NPP, (ch + 1) * NPP)
                    # ---- index path: [16, 1024] compact wrapped layout,
                    # bit-identical floor chain to axis_prep ----
                    xi = ipool.tile([16, NPP], f32, tag="xi")
                    yi = ipool.tile([16, NPP], f32, tag="yi")
                    nc.sync.dma_start(xi[:], xi_t[:, ws])
                    nc.sync.dma_start(yi[:], yi_t[:, ws])
                    qx = ipool.tile([16, NPP], i32, tag="qx")
                    hy = ipool.tile([16, NPP], i32, tag="hy")
                    v.tensor_scalar(xi[:], xi[:], INV_BSX, None, AL.mult)
                    v.tensor_copy(qx[:], xi[:])   # f32->i32 trunc == floor
                    v.tensor_scalar(qx[:], qx[:], 2, None,
                                    AL.arith_shift_right)
                    v.tensor_scalar(yi[:], yi[:], INV_BSY, None, AL.mult)
                    v.tensor_copy(hy[:], yi[:])
                    v.tensor_scalar(hy[:], hy[:], 1, None,
                                    AL.arith_shift_right)
                    idx16 = ipool.tile([16, NPP], i16, tag="idx16")
                    v.scalar_tensor_tensor(out=idx16[:], in0=qx[:], scalar=256,
                                           in1=hy[:], op0=AL.mult, op1=AL.add)
                    idxt = ipool.tile([P, NPP], i16, tag="idxt")
                    # replicate the 16-partition wrapped indices to all 8
                    # GPSIMD core groups
                    for g in range(8):
                        nc.sync.dma_start(idxt[16 * g:16 * (g + 1), :],
                                          idx16[:])

                    # ---- gather: one 24-fp16 record per node ----
                    w = wpool.tile([P, CHUNK * ESIZE], f16, tag="w")
                    for j in range(NSUB):
                        _emit_dma_gather(
                            nc,
                            w[:, j * SUBC * ESIZE:(j + 1) * SUBC * ESIZE]
                            .rearrange("p (c e) -> p c e", e=ESIZE),
                            wt_gather_ap,
                            idxt[:, j * SUBC * 8:(j + 1) * SUBC * 8],
                            SUBC * P, ESIZE, ESTEP, queue_num=j % 4,
                        )

                    # ---- reduce: area = (ovx (x) ovy) . W * (BSX*BSY) ----
                    m = w[:].rearrange("p (c a b) -> p c a b", a=NTAPX,
                                       b=NTAPY)
                    ovy_b = ovy[:, cs].unsqueeze(2).to_broadcast(
                        [P, CHUNK, NTAPX, NTAPY])
                    v.tensor_tensor(m, m, ovy_b, AL.mult)
                    t = rpool.tile([P, CHUNK, NTAPX], f16, tag="t")
                    with nc.allow_low_precision(
                            reason="fp16 4-tap partial sums; tol is 2e-2"):
                        v.tensor_reduce(t[:], m, AX.X, AL.add)
                    v.tensor_tensor(t[:], t[:], ovx[:, cs], AL.mult)
                    area = rpool.tile([P, CHUNK], f32, tag="area")
                    v.tensor_reduce(area[:], t[:], AX.X, AL.add)
                    v.tensor_scalar(area[:], area[:], BSX * BSY, None, AL.mult)
                    nc.sync.dma_start(out_t[:, cs], area[:])

            if repeat == 1:
                body()
            else:
                with tc.For_i(0, repeat, 1):
                    body()

    nc.compile()
    return nc


def make_window_table(utilization_map):
    U = np.asarray(utilization_map, np.float32)
    # tap offsets start at -1 (base may be floor+1); pad a zero row/col
    # in front and clamp-pad the top
    Upad = np.zeros((520, 520), np.float32)
    Upad[1:513, 1:513] = U
    # WT[qx*256+hy, a*5+b] = U[4qx + (a-1), 2hy + (b-1)]
    a = np.arange(NTAPX)
    b = np.arange(NTAPY)
    qx = np.arange(128)
    hy = np.arange(256)
    rows = (4 * qx[:, None, None, None] + a[None, None, :, None])
    cols = (2 * hy[None, :, None, None] + b[None, None, None, :])
    win = Upad[rows, cols]                                  # [128,256,7,5]
    wt = np.zeros((NW, ESTEP), np.float16)
    wt[:, :ESIZE] = win.reshape(NW, ESIZE).astype(np.float16)
    return wt


def make_in_maps(pos, node_size_x, node_size_y, utilization_map):
    n = NUM_MOVABLE
    half = pos.shape[0] // 2
    x = np.asarray(pos[:n], np.float32)
    y = np.asarray(pos[half:half + n], np.float32)
    sx = np.asarray(node_size_x, np.float32)
    sy = np.asarray(node_size_y, np.float32)

    tot = NCORES * NPC
    xp = np.full(tot, 500.0, np.float32)
    yp = np.full(tot, 500.0, np.float32)
    sxp = np.full(tot, 0.5, np.float32)
    syp = np.full(tot, 0.5, np.float32)
    xp[:n] = x
    yp[:n] = y
    sxp[:n] = sx
    syp[:n] = sy

    wt = make_window_table(utilization_map)

    def transp(arr_core):
        # node i = c*128 + p  ->  DRAM p-major: out[p*1024 + c]
        return arr_core.reshape(NPP, P).T.copy().reshape(-1)

    def wrapped16(arr_core):
        # value for idx slot (r, s = cm*8 + a) = arr[cm*128 + 16a + r]
        v = arr_core.reshape(NPP, 8, 16)          # [cm, a, r]
        return v.transpose(2, 0, 1).copy().reshape(-1)

    in_maps = []
    for k in range(NCORES):
        s = slice(k * NPC, (k + 1) * NPC)
        in_maps.append(dict(x_in=transp(xp[s]), y_in=transp(yp[s]),
                            sx_in=transp(sxp[s]), sy_in=transp(syp[s]),
                            xi_in=wrapped16(xp[s]), yi_in=wrapped16(yp[s]),
                            wt_in=wt))
    return in_maps


def unshard_area(outs):
    """Per-core p-major [NPC] arrays -> flat node order."""
    res = []
    for o in outs:
        res.append(np.asarray(o).reshape(P, NPP).T.reshape(-1))
    return np.concatenate(res)


_NC_CACHE = {}


def _get_nc(repeat=1):
    if repeat not in _NC_CACHE:
        _NC_CACHE[repeat] = build(repeat)
    return _NC_CACHE[repeat]


def kernel(pos, node_size_x, node_size_y, utilization_map):
    in_maps = make_in_maps(pos, node_size_x, node_size_y, utilization_map)
    nc = _get_nc(1)
    res = bass_utils.run_bass_kernel_spmd(nc, in_maps,
                                          core_ids=list(range(NCORES)))
    area = unshard_area([r["area_out"] for r in res.results])[:NUM_MOVABLE]
    return area.astype(np.float32)
